# revision 11
# baseline (speedup 1.0000x reference)
"""BiMamba (bidirectional Mamba block + LN + FFN) Trainium2 Bass kernel.

Sharding (8 cores): 4 scan-sequences (fwd/bwd x batch, bwd fed host-flipped x)
x 2 halves of d_inner. Device layout is feature-on-partitions /
time-on-free throughout; the host transposes x on the way in and the output
on the way out. Cross-core combines (out_proj partial sums + direction
merge, ff2 partial sums) use AllGather/AllReduce over quads
[0,1,4,5] / [2,3,6,7].
"""
import sys, os, types, contextlib, ctypes

sys.path.insert(0, "/opt/trn_rl_repo")
import numpy as np

D_MODEL = 1024
D_STATE = 16
D_CONV = 4
D_INNER = 2048
DT_RANK = 64
L = 1024
HALF = D_INNER // 2          # 1024 d_inner per core
P = 128
NJ = HALF // P               # 8 d-blocks per core half
TCH = 512                    # matmul t-chunk
NT = L // TCH
KD = D_MODEL // P            # 8 k-chunks over d_model
NFB_XC = D_INNER // P        # 16 xc feature blocks (full d_inner)
FF_SLICE = 1024              # ffn hidden slice per core
NB = DT_RANK + 2 * D_STATE   # 96

_GROUPS = [[0, 1, 4, 5], [2, 3, 6, 7]]


def _install_ntff_hook_shim(so_path="/opt/axon/libaxon_pjrt.so"):
    if "antenv.axon_hooks" in sys.modules:
        return
    try:
        lib = ctypes.CDLL(so_path)
    except OSError:
        return
    if not hasattr(lib, "axon_start_nrt_profile"):
        return
    lib.axon_start_nrt_profile.argtypes = [ctypes.POINTER(ctypes.c_int64), ctypes.c_size_t]
    lib.axon_start_nrt_profile.restype = ctypes.c_int64
    lib.axon_stop_nrt_profile.argtypes = [ctypes.c_char_p]
    lib.axon_stop_nrt_profile.restype = ctypes.c_int64

    @contextlib.contextmanager
    def _hook(output_dir, device_ids):
        import jax
        jax.devices()
        if device_ids:
            ids = (ctypes.c_int64 * len(device_ids))(*device_ids)
            rc = lib.axon_start_nrt_profile(ids, len(device_ids))
        else:
            rc = lib.axon_start_nrt_profile(None, 0)
        if rc != 0:
            raise RuntimeError(f"axon_start_nrt_profile rc={rc}")
        try:
            yield
        finally:
            n = lib.axon_stop_nrt_profile(str(output_dir).encode())
            print(f"profile: {n} file(s) written to {output_dir}", file=sys.stderr)

    mod = types.ModuleType("antenv.axon_hooks")
    mod.get_axon_ntff_profile_hook = lambda: _hook
    mod.set_axon_ntff_profile_hook = lambda h: None
    sys.modules["antenv.axon_hooks"] = mod


def _build_nc():
    from concourse import bacc, tile, mybir

    f32 = mybir.dt.float32
    f32r = mybir.dt.float32r
    bf16 = mybir.dt.bfloat16
    Alu = mybir.AluOpType
    Act = mybir.ActivationFunctionType

    def r(ap):
        return ap.bitcast(f32r)

    nc = bacc.Bacc("TRN2", target_bir_lowering=False, debug=False, num_devices=8)

    def din(name, shape, dt=None):
        return nc.dram_tensor(name, list(shape), dt or f32, kind="ExternalInput").ap()

    xT = din("xT", (D_MODEL, L), f32r)
    w_in_t = din("w_in_t", (NJ + NFB_XC, KD, P, P), f32r)          # z-half blocks, then xc
    conv_diag = din("conv_diag", (NFB_XC, D_CONV, P, P), f32r)
    convb_cols = din("convb_cols", (P, NFB_XC))
    xpw_t = din("xpw_t", (NFB_XC, P, P), f32r)  # cols: dt64|B16|pad|C16|pad
    dtw_t = din("dtw_t", (NJ, DT_RANK, P), f32r)
    dtb_cols = din("dtb_cols", (P, NJ))
    A_cols = din("A_cols", (P, NJ * D_STATE))
    D_colsT = din("D_colsT", (P, NJ))
    outw_t = din("outw_t", (KD, NJ, P, P), f32r)                   # [k(d_in), m(dm)]
    lng_cols = din("lng_cols", (P, KD))
    lnb_cols = din("lnb_cols", (P, KD))
    w1_t = din("w1_t", (KD, NJ, P, P), f32r)                       # [k(dm), m(h)]
    b1_cols = din("b1_cols", (P, NJ))
    w2_t = din("w2_t", (NJ, KD, P, P), f32r)                       # [k(h), m(dm)]
    b2_cols = din("b2_cols", (P, KD))
    consts_r = din("consts_r", (P, 4), f32r)  # col0=1/1024, cols1..3=0

    out_m = nc.dram_tensor("out_m", [D_MODEL, L], f32, kind="ExternalOutput").ap()

    es = contextlib.ExitStack()

    with tile.TileContext(nc) as tc:
        with contextlib.ExitStack() as stk:
            cpool = stk.enter_context(tc.tile_pool(name="cpool", bufs=1))
            psum = stk.enter_context(tc.tile_pool(name="psum", bufs=4, space="PSUM"))
            dram = stk.enter_context(tc.tile_pool(name="dram", bufs=1, space="DRAM"))

            def cload(src, shape, tag):
                t = cpool.tile(list(shape), f32, tag=tag, name=tag)
                nc.sync.dma_start(t[:], src)
                return t

            A_sb = cload(A_cols[:], (P, NJ * D_STATE), "A_sb")
            dtb_sb = cload(dtb_cols[:], (P, NJ), "dtb_sb")
            D_sb = cload(D_colsT[:], (P, NJ), "D_sb")
            convb_sb = cload(convb_cols[:], (P, NFB_XC), "convb_sb")
            lng_sb = cload(lng_cols[:], (P, KD), "lng_sb")
            lnb_sb = cload(lnb_cols[:], (P, KD), "lnb_sb")
            b1_sb = cload(b1_cols[:], (P, NJ), "b1_sb")
            b2_sb = cload(b2_cols[:], (P, KD), "b2_sb")
            ones_sb = cpool.tile([P, 1], f32r, tag="ones_sb", name="ones_sb")
            nc.sync.dma_start(ones_sb[:], consts_r[:, 0:1])

            bcB_dram = dram.tile([D_STATE, L], bf16, name="bcB_dram")
            bcC_dram = dram.tile([D_STATE, L], f32, name="bcC_dram")
            stat_dram = dram.tile([2, L], f32, name="stat_dram")
            ag_in = dram.tile([D_MODEL, L], f32, name="ag_in")
            ag_out = dram.tile([4 * D_MODEL, L], f32, name="ag_out")
            ar_in = dram.tile([D_MODEL, L], f32, name="ar_in")
            ar_out = dram.tile([D_MODEL, L], f32, name="ar_out")

            def mm_accum(ps, lw_list, rhs_of_k, n_k):
                for k in range(n_k):
                    nc.tensor.matmul(ps[:], lw_list[k][:], rhs_of_k(k),
                                     start=(k == 0), stop=(k == n_k - 1))

            # =========== P1-P4 region: sz lives until gating ===========
            with tc.tile_pool(name="sz_pool", bufs=1) as sz_pool:
                sz = [sz_pool.tile([P, L], f32, tag=f"sz{j}", name=f"sz{j}")
                      for j in range(NJ)]
                dt_sb = sz_pool.tile([DT_RANK, L], f32r, tag="dt_sb", name="dt_sb")

                def dt_proj_delta(j, delta_t, pool):
                    # softplus(x + b) = Ln(1 + Exp(x + b)); inputs here are
                    # well below 0 so Exp cannot overflow.
                    lw = pool.tile([DT_RANK, P], f32r, tag="dtw", name=f"dtw{j}", bufs=2)
                    nc.sync.dma_start(lw[:], dtw_t[j])
                    for t in range(NT):
                        ps = psum.tile([P, TCH], f32, tag="ps", name=f"dtp{j}_{t}")
                        nc.tensor.matmul(ps[:], lw[:],
                                         dt_sb[:, t * TCH:(t + 1) * TCH],
                                         start=True, stop=True)
                        spt = pool.tile([P, TCH], f32, tag="spt", name=f"spt{j}_{t}",
                                        bufs=2)
                        nc.scalar.activation(spt[:], ps[:], Act.Exp,
                                             bias=dtb_sb[:, j:j + 1])
                        nc.scalar.activation(delta_t[:, t * TCH:(t + 1) * TCH], spt[:],
                                             Act.Ln, bias=1.0)

                # ---------------- P1..P3: need xc blocks ----------------
                with tc.tile_pool(name="xc_pool", bufs=1) as xc_pool:
                    xcs = [xc_pool.tile([P, L], f32r, tag=f"xcs{j}", name=f"xcs{j}")
                           for j in range(NFB_XC)]

                    # P1: in_proj + conv + silu
                    with tc.tile_pool(name="xt_pool", bufs=1) as xt_pool, \
                         tc.tile_pool(name="p1t", bufs=1) as p1t:
                        xts = []
                        for k in range(KD):
                            xt_k = xt_pool.tile([P, L], f32r, tag=f"xt{k}", name=f"xt{k}")
                            nc.sync.dma_start(xt_k[:], xT[k * P:(k + 1) * P, :])
                            xts.append(xt_k)

                        def in_proj_block(fb):
                            lws = []
                            for k in range(KD):
                                lw = p1t.tile([P, P], f32r, tag=f"lw{k}",
                                              name=f"lw{fb}_{k}", bufs=2)
                                nc.sync.dma_start(lw[:], w_in_t[fb, k])
                                lws.append(lw)
                            pss = []
                            for t in range(NT):
                                ps = psum.tile([P, TCH], f32, tag="ps",
                                               name=f"inp{fb}_{t}")
                                mm_accum(ps, lws,
                                         lambda k: xts[k][:, t * TCH:(t + 1) * TCH], KD)
                                pss.append(ps)
                            return pss

                        for j in range(NJ):  # z half
                            for t, ps in enumerate(in_proj_block(j)):
                                nc.scalar.activation(sz[j][:, t * TCH:(t + 1) * TCH],
                                                     ps[:], Act.Silu)

                        for j in range(NFB_XC):  # xc blocks + conv
                            xcp = p1t.tile([P, L + D_CONV - 1], f32r, tag="xcp",
                                           name=f"xcp{j}", bufs=2)
                            nc.sync.dma_start(xcp[:, 0:D_CONV - 1], consts_r[:, 1:D_CONV])
                            for t, ps in enumerate(in_proj_block(NJ + j)):
                                nc.scalar.copy(
                                    xcp[:, D_CONV - 1 + t * TCH:D_CONV - 1 + (t + 1) * TCH],
                                    ps[:])
                            cds = []
                            for i in range(D_CONV):
                                cd = p1t.tile([P, P], f32r, tag=f"cd{i}",
                                              name=f"cd{j}_{i}", bufs=2)
                                nc.sync.dma_start(cd[:], conv_diag[j, i])
                                cds.append(cd)
                            for t in range(NT):
                                cps = psum.tile([P, TCH], f32, tag="ps",
                                                name=f"cps{j}_{t}")
                                mm_accum(cps, cds,
                                         lambda i: xcp[:, t * TCH + i:t * TCH + i + TCH],
                                         D_CONV)
                                nc.scalar.activation(xcs[j][:, t * TCH:(t + 1) * TCH],
                                                     cps[:], Act.Silu,
                                                     bias=convb_sb[:, j:j + 1])

                    # right-side pools for wv/g0 (live P2..P4)
                    wvg_ctx = contextlib.ExitStack()
                    wv_pool = wvg_ctx.enter_context(
                        tc.tile_pool(name="wv_pool", bufs=1, side="right"))
                    wvs = [wv_pool.tile([P, L], bf16, tag=f"wv{j}", name=f"wv{j}")
                           for j in range(NJ)]
                    g0_pool = wvg_ctx.enter_context(
                        tc.tile_pool(name="g0_pool", bufs=1, side="right"))
                    g0s = [g0_pool.tile([P, L], f32, tag=f"g0{j}", name=f"g0{j}")
                           for j in range(NJ)]

                    # P2: x_proj; P3: wv/g0
                    with tc.tile_pool(name="p2t", bufs=1) as p2t:
                        for t in range(NT):
                            ps = psum.tile([P, TCH], f32, tag="ps", name=f"xproj{t}")
                            for k in range(NFB_XC):
                                lw = p2t.tile([P, P], f32r, tag="xpw",
                                              name=f"xpw{t}_{k}", bufs=2)
                                nc.sync.dma_start(lw[:], xpw_t[k])
                                nc.tensor.matmul(ps[:], lw[:],
                                                 xcs[k][:, t * TCH:(t + 1) * TCH],
                                                 start=(k == 0), stop=(k == NFB_XC - 1))
                            nc.scalar.copy(dt_sb[:, t * TCH:(t + 1) * TCH],
                                           ps[0:DT_RANK, :])
                            bcB_sb = p2t.tile([D_STATE, TCH], bf16, tag="bcB_sb",
                                              name=f"bcB_sb{t}", bufs=2)
                            nc.scalar.copy(bcB_sb[:], ps[64:80, :])
                            nc.sync.dma_start(bcB_dram[:, t * TCH:(t + 1) * TCH], bcB_sb[:])
                            bcC_sb = p2t.tile([D_STATE, TCH], f32, tag="bcC_sb",
                                              name=f"bcC_sb{t}", bufs=2)
                            nc.scalar.copy(bcC_sb[:], ps[96:112, :])
                            nc.sync.dma_start(bcC_dram[:, t * TCH:(t + 1) * TCH], bcC_sb[:])

                        for j in range(NJ):
                            dtmp = p2t.tile([P, L], f32, tag="dtmp", name=f"dtmp{j}",
                                            bufs=2)
                            dt_proj_delta(j, dtmp, p2t)
                            nc.vector.tensor_tensor(wvs[j][:], dtmp[:], xcs[j][:],
                                                    Alu.mult)
                            t1 = p2t.tile([P, L], f32, tag="g0tmp", name=f"g0tmp{j}",
                                          bufs=2)
                            nc.vector.tensor_scalar_mul(t1[:], xcs[j][:], D_sb[:, j:j + 1])
                            nc.vector.tensor_tensor(g0s[j][:], t1[:], sz[j][:], Alu.mult)

                # ---------------- P4: scan + gating ----------------
                yg_ctx = contextlib.ExitStack()
                yg_pool = yg_ctx.enter_context(tc.tile_pool(name="yg_pool", bufs=1))
                ygs = [yg_pool.tile([P, L], f32r, tag=f"yg{j}", name=f"yg{j}")
                       for j in range(NJ)]
                with tc.tile_pool(name="spool", bufs=1) as spool, \
                     tc.tile_pool(name="tpool", bufs=1) as tpool:
                    for hb in range(2):
                        js = list(range(hb * 4, hb * 4 + 4))
                        deltas = {}
                        ys = {}
                        for j in js:
                            dj = spool.tile([P, L], f32, tag=f"delta{j - hb * 4}",
                                            name=f"delta{j}")
                            dt_proj_delta(j, dj, spool)
                            deltas[j] = dj
                            ys[j] = spool.tile([P, L], f32, tag=f"y{j - hb * 4}",
                                               name=f"y{j}")
                        for n in range(D_STATE):
                            Bbc = tpool.tile([P, L], bf16, tag="Bbc",
                                             name=f"Bbc{hb}_{n}", bufs=2)
                            nc.sync.dma_start(
                                Bbc[:],
                                bcB_dram[n:n + 1, :].partition_broadcast(P).squeeze(1))
                            Cbc = tpool.tile([P, L], f32, tag="Cbc",
                                             name=f"Cbc{hb}_{n}", bufs=2)
                            nc.sync.dma_start(
                                Cbc[:],
                                bcC_dram[n:n + 1, :].partition_broadcast(P).squeeze(1))
                            for j in js:
                                a_t = tpool.tile([P, L], f32, tag="a_t",
                                                 name=f"a{j}_{n}", bufs=2)
                                nc.scalar.activation(
                                    a_t[:], deltas[j][:], Act.Exp,
                                    scale=A_sb[:, j * D_STATE + n:j * D_STATE + n + 1])
                                b_t = tpool.tile([P, L], bf16, tag="b_t",
                                                 name=f"b{j}_{n}", bufs=1)
                                nc.vector.tensor_tensor(b_t[:], wvs[j][:], Bbc[:],
                                                        Alu.mult)
                                h_t = tpool.tile([P, L], f32, tag="h_t",
                                                 name=f"h{j}_{n}", bufs=1)
                                nc.vector.tensor_tensor_scan(h_t[:], a_t[:], b_t[:],
                                                             0.0, Alu.mult, Alu.add)
                                if n == 0:
                                    nc.vector.tensor_tensor(ys[j][:], h_t[:], Cbc[:],
                                                            Alu.mult)
                                else:
                                    prod = tpool.tile([P, L], f32, tag="prod",
                                                      name=f"p{j}_{n}", bufs=2)
                                    peng = nc.vector if (n % 2) else nc.gpsimd
                                    peng.tensor_tensor(prod[:], h_t[:], Cbc[:],
                                                       Alu.mult)
                                    nc.gpsimd.tensor_tensor(ys[j][:], ys[j][:],
                                                            prod[:], Alu.add)
                        for j in js:
                            nc.vector.tensor_tensor(ygs[j][:], ys[j][:], sz[j][:],
                                                    Alu.mult)
                            nc.vector.tensor_tensor(ygs[j][:], ygs[j][:], g0s[j][:],
                                                    Alu.add)
                wvg_ctx.close()  # wv + g0 (right side, LIFO: g0 then wv)

                # =========== P5: out_proj partials ===========
                with tc.tile_pool(name="p5t", bufs=1) as p5t:
                    for m in range(NJ):
                        lws = []
                        for k in range(KD):
                            lw = p5t.tile([P, P], f32r, tag=f"lw{k}",
                                          name=f"ow{m}_{k}", bufs=2)
                            nc.sync.dma_start(lw[:], outw_t[k, m])
                            lws.append(lw)
                        msb = p5t.tile([P, L], f32, tag="msb", name=f"msb{m}", bufs=2)
                        for t in range(NT):
                            ps = psum.tile([P, TCH], f32, tag="ps", name=f"op{m}_{t}")
                            mm_accum(ps, lws,
                                     lambda k: ygs[k][:, t * TCH:(t + 1) * TCH], KD)
                            nc.scalar.copy(msb[:, t * TCH:(t + 1) * TCH], ps[:])
                        nc.sync.dma_start(ag_in[m * P:(m + 1) * P, :], msb[:])
                yg_ctx.close()

            nc.gpsimd.collective_compute("AllGather", Alu.bypass,
                                         replica_groups=_GROUPS,
                                         ins=[ag_in[:]], outs=[ag_out[:]])

            # =========== P6: mo + LN ===========
            with contextlib.ExitStack() as stk2:
                mo_pool = stk2.enter_context(tc.tile_pool(name="mo_pool", bufs=1))
                mos = [mo_pool.tile([P, L], f32r, tag=f"mo{j}", name=f"mo{j}")
                       for j in range(KD)]
                xn_pool = stk2.enter_context(tc.tile_pool(name="xn_pool", bufs=1))
                xns = [xn_pool.tile([P, L], f32r, tag=f"xn{j}", name=f"xn{j}")
                       for j in range(KD)]

                with tc.tile_pool(name="p6t", bufs=1) as p6t:
                    mu_ps = psum.tile([1, L], f32, tag="mu_ps", name="mu_ps", bufs=1)
                    e2_ps = psum.tile([1, L], f32, tag="e2_ps", name="e2_ps", bufs=1)
                    for j in range(KD):
                        parts = []
                        for q in range(4):
                            pt = p6t.tile([P, L], f32, tag="agp", name=f"agp{j}_{q}",
                                          bufs=4)
                            nc.sync.dma_start(
                                pt[:],
                                ag_out[q * D_MODEL + j * P:q * D_MODEL + (j + 1) * P, :])
                            parts.append(pt)
                        a01 = p6t.tile([P, L], f32, tag="a01", name=f"a01_{j}", bufs=2)
                        nc.vector.tensor_tensor(a01[:], parts[0][:], parts[1][:],
                                                Alu.add)
                        a23 = p6t.tile([P, L], f32, tag="a23", name=f"a23_{j}", bufs=2)
                        nc.vector.tensor_tensor(a23[:], parts[2][:], parts[3][:],
                                                Alu.add)
                        nc.vector.tensor_tensor(mos[j][:], a01[:], a23[:, ::-1],
                                                Alu.add)
                        sq = p6t.tile([P, L], f32r, tag="sq", name=f"sq{j}", bufs=2)
                        nc.scalar.activation(sq[:], mos[j][:], Act.Square)
                        for t in range(NT):
                            sl = slice(t * TCH, (t + 1) * TCH)
                            nc.tensor.matmul(mu_ps[:, sl], ones_sb[:],
                                             mos[j][:, sl],
                                             start=(j == 0), stop=(j == KD - 1))
                            nc.tensor.matmul(e2_ps[:, sl], ones_sb[:],
                                             sq[:, sl],
                                             start=(j == 0), stop=(j == KD - 1))

                    mean_sb = p6t.tile([1, L], f32, tag="mean_sb", name="mean_sb",
                                       bufs=1)
                    nc.scalar.copy(mean_sb[:], mu_ps[:])
                    m2 = p6t.tile([1, L], f32, tag="m2", name="m2", bufs=1)
                    nc.vector.tensor_tensor(m2[:], mean_sb[:], mean_sb[:], Alu.mult)
                    var_t = p6t.tile([1, L], f32, tag="var_t", name="var_t", bufs=1)
                    nc.vector.tensor_tensor(var_t[:], e2_ps[:], m2[:], Alu.subtract)
                    eps_sb = p6t.tile([1, 1], f32, tag="eps_sb", name="eps_sb", bufs=1)
                    nc.vector.memset(eps_sb[:], 1e-5)
                    std_t = p6t.tile([1, L], f32, tag="std_t", name="std_t", bufs=1)
                    nc.scalar.activation(std_t[:], var_t[:], Act.Sqrt, bias=eps_sb[:])
                    rstd_sb = p6t.tile([1, L], f32, tag="rstd_sb", name="rstd_sb",
                                       bufs=1)
                    nc.vector.reciprocal(rstd_sb[:], std_t[:])
                    nc.sync.dma_start(stat_dram[0:1, :], mean_sb[:])
                    nc.sync.dma_start(stat_dram[1:2, :], rstd_sb[:])
                    mean_bc = p6t.tile([P, L], f32, tag="mean_bc", name="mean_bc",
                                       bufs=1)
                    nc.sync.dma_start(
                        mean_bc[:],
                        stat_dram[0:1, :].partition_broadcast(P).squeeze(1))
                    rstd_bc = p6t.tile([P, L], f32, tag="rstd_bc", name="rstd_bc",
                                       bufs=1)
                    nc.sync.dma_start(
                        rstd_bc[:],
                        stat_dram[1:2, :].partition_broadcast(P).squeeze(1))

                    for j in range(KD):
                        t1 = p6t.tile([P, L], f32, tag="lnt", name=f"lnt{j}", bufs=2)
                        nc.vector.tensor_tensor(t1[:], mos[j][:], mean_bc[:],
                                                Alu.subtract)
                        nc.vector.tensor_tensor(t1[:], t1[:], rstd_bc[:], Alu.mult)
                        nc.vector.tensor_scalar(xns[j][:], t1[:], lng_sb[:, j:j + 1],
                                                lnb_sb[:, j:j + 1], Alu.mult, Alu.add)

                # =========== P7: FFN ===========
                with tc.tile_pool(name="ffh_pool", bufs=1) as ffh_pool, \
                     tc.tile_pool(name="p7t", bufs=1) as p7t:
                    ffhs = [ffh_pool.tile([P, L], f32r, tag=f"ffh{m}", name=f"ffh{m}")
                            for m in range(NJ)]
                    for m in range(NJ):
                        lws = []
                        for k in range(KD):
                            lw = p7t.tile([P, P], f32r, tag=f"lw{k}", name=f"w1_{m}_{k}",
                                          bufs=2)
                            nc.sync.dma_start(lw[:], w1_t[k, m])
                            lws.append(lw)
                        for t in range(NT):
                            ps = psum.tile([P, TCH], f32, tag="ps", name=f"f1{m}_{t}")
                            mm_accum(ps, lws,
                                     lambda k: xns[k][:, t * TCH:(t + 1) * TCH], KD)
                            nc.scalar.activation(ffhs[m][:, t * TCH:(t + 1) * TCH],
                                                 ps[:], Act.Gelu,
                                                 bias=b1_sb[:, m:m + 1])

                    for m in range(KD):
                        lws = []
                        for k in range(NJ):
                            lw = p7t.tile([P, P], f32r, tag=f"lw{k}", name=f"w2_{m}_{k}",
                                          bufs=2)
                            nc.sync.dma_start(lw[:], w2_t[k, m])
                            lws.append(lw)
                        msb = p7t.tile([P, L], f32, tag="msb", name=f"f2sb{m}", bufs=2)
                        for t in range(NT):
                            ps = psum.tile([P, TCH], f32, tag="ps", name=f"f2{m}_{t}")
                            mm_accum(ps, lws,
                                     lambda k: ffhs[k][:, t * TCH:(t + 1) * TCH], NJ)
                            nc.scalar.copy(msb[:, t * TCH:(t + 1) * TCH], ps[:])
                        nc.sync.dma_start(ar_in[m * P:(m + 1) * P, :], msb[:])

            nc.gpsimd.collective_compute("AllReduce", Alu.add, replica_groups=_GROUPS,
                                         ins=[ar_in[:]], outs=[ar_out[:]])

            with tc.tile_pool(name="p8t", bufs=1) as p8t:
                for j in range(KD):
                    fin = p8t.tile([P, L], f32, tag="fin", name=f"fin{j}", bufs=2)
                    nc.sync.dma_start(fin[:], ar_out[j * P:(j + 1) * P, :])
                    fob = p8t.tile([P, L], f32, tag="fob", name=f"fob{j}", bufs=2)
                    nc.vector.tensor_scalar_add(fob[:], fin[:], b2_sb[:, j:j + 1])
                    nc.sync.dma_start(out_m[j * P:(j + 1) * P, :], fob[:])

    nc.compile()
    return nc


def _prep_inputs(inputs):
    """Per-core input dicts. Core c: sequence s=c//2 (s>=2 => time-flipped x),
    d_inner half = c%2. The own half of d_inner is permuted FIRST in every
    d_inner-ordered tensor, so the device kernel is identical on all cores."""
    x = np.asarray(inputs["x"], dtype=np.float32)
    in_proj_w = np.asarray(inputs["in_proj_w"], dtype=np.float32)
    conv_w = np.asarray(inputs["conv_w"], dtype=np.float32)
    conv_b = np.asarray(inputs["conv_b"], dtype=np.float32)
    x_proj_w = np.asarray(inputs["x_proj_w"], dtype=np.float32)
    dt_proj_w = np.asarray(inputs["dt_proj_w"], dtype=np.float32)
    dt_proj_b = np.asarray(inputs["dt_proj_b"], dtype=np.float32)
    A = -np.exp(np.asarray(inputs["A_log"], dtype=np.float32))
    Dp = np.asarray(inputs["D"], dtype=np.float32)
    out_proj_w = np.asarray(inputs["out_proj_w"], dtype=np.float32)
    ln_g = np.asarray(inputs["ln_g"], dtype=np.float32)
    ln_b = np.asarray(inputs["ln_b"], dtype=np.float32)
    ff_w1 = np.asarray(inputs["ff_w1"], dtype=np.float32)
    ff_b1 = np.asarray(inputs["ff_b1"], dtype=np.float32)
    ff_w2 = np.asarray(inputs["ff_w2"], dtype=np.float32)
    ff_b2 = np.asarray(inputs["ff_b2"], dtype=np.float32)

    def cols(v):  # (N,) -> (P, N//P) per-partition column layout
        return np.ascontiguousarray(v.reshape(-1, P).T)

    def tile_w(w, KP, MP):  # (K, M) -> (K//KP, M//MP, KP, MP)
        K, M = w.shape
        return np.ascontiguousarray(
            w.reshape(K // KP, KP, M // MP, MP).transpose(0, 2, 1, 3))

    in_maps = []
    for c in range(8):
        s, half = c // 2, c % 2
        xb = x[s] if s < 2 else x[s - 2][::-1]
        perm = np.arange(D_INNER).reshape(2, HALF)
        perm = np.concatenate([perm[half], perm[1 - half]])
        own = perm[:HALF]

        wz = in_proj_w[:, D_INNER + own]                      # (1024, 1024)
        wxc = in_proj_w[:, perm]                              # (1024, 2048)
        w_in = np.concatenate([wz, wxc], axis=1)              # (1024, 3072)
        w_in_t = np.ascontiguousarray(tile_w(w_in, P, P).transpose(1, 0, 2, 3))

        cw = conv_w[perm]
        conv_diag = np.zeros((NFB_XC, D_CONV, P, P), np.float32)
        idx = np.arange(P)
        for j in range(NFB_XC):
            for i in range(D_CONV):
                conv_diag[j, i, idx, idx] = cw[j * P:(j + 1) * P, i]

        g = (c & 1) + 2 * (c >> 2)
        hsl = slice(g * FF_SLICE, (g + 1) * FF_SLICE)

        in_maps.append({
            "xT": np.ascontiguousarray(xb.T),
            "w_in_t": w_in_t,
            "conv_diag": conv_diag,
            "convb_cols": cols(conv_b[perm]),
            "xpw_t": np.ascontiguousarray(
                np.concatenate([
                    x_proj_w[perm][:, :DT_RANK + D_STATE],
                    np.zeros((D_INNER, D_STATE), np.float32),
                    x_proj_w[perm][:, DT_RANK + D_STATE:],
                    np.zeros((D_INNER, D_STATE), np.float32),
                ], axis=1).reshape(NFB_XC, P, P)),
            "dtw_t": np.ascontiguousarray(
                dt_proj_w[:, own].reshape(DT_RANK, NJ, P).transpose(1, 0, 2)),
            "dtb_cols": cols(dt_proj_b[own]),
            "A_cols": np.ascontiguousarray(
                A[own].reshape(NJ, P, D_STATE).transpose(1, 0, 2).reshape(P, NJ * D_STATE)),
            "D_colsT": cols(Dp[own]),
            "outw_t": tile_w(out_proj_w[own], P, P),
            "lng_cols": cols(ln_g),
            "lnb_cols": cols(ln_b),
            "w1_t": tile_w(ff_w1[:, hsl], P, P),
            "b1_cols": cols(ff_b1[hsl]),
            "w2_t": tile_w(ff_w2[hsl], P, P),
            "b2_cols": cols(ff_b2),
            "consts_r": np.concatenate(
                [np.full((P, 1), 1.0 / D_MODEL, np.float32),
                 np.zeros((P, 3), np.float32)], axis=1),
        })
    return in_maps


_NC_CACHE = {}


def _get_nc():
    if "nc" not in _NC_CACHE:
        _NC_CACHE["nc"] = _build_nc()
    return _NC_CACHE["nc"]


def run(inputs, trace=False):
    _install_ntff_hook_shim()
    from concourse import bass_utils
    nc = _get_nc()
    in_maps = _prep_inputs(inputs)
    res = bass_utils.run_bass_kernel_spmd(nc, in_maps, core_ids=list(range(8)),
                                          trace=trace)
    out = np.stack([
        np.ascontiguousarray(res.results[0]["out_m"].T),
        np.ascontiguousarray(res.results[2]["out_m"].T),
    ]).astype(np.float32)
    return out, res


def kernel(**inputs):
    out, _ = run(inputs, trace=False)
    return out


# revision 12
# speedup vs baseline: 1.1680x; 1.1680x over previous
"""BiMamba (bidirectional Mamba block + LN + FFN) Trainium2 Bass kernel.

Sharding (8 cores): 4 scan-sequences (fwd/bwd x batch, bwd fed host-flipped x)
x 2 halves of d_inner. Device layout is feature-on-partitions /
time-on-free throughout; the host transposes x on the way in and the output
on the way out. Cross-core combines (out_proj partial sums + direction
merge, ff2 partial sums) use AllGather/AllReduce over quads
[0,1,4,5] / [2,3,6,7].
"""
import sys, os, types, contextlib, ctypes

sys.path.insert(0, "/opt/trn_rl_repo")
import numpy as np

D_MODEL = 1024
D_STATE = 16
D_CONV = 4
D_INNER = 2048
DT_RANK = 64
L = 1024
HALF = D_INNER // 2          # 1024 d_inner per core
P = 128
NJ = HALF // P               # 8 d-blocks per core half
TCH = 512                    # matmul t-chunk
NT = L // TCH
KD = D_MODEL // P            # 8 k-chunks over d_model
NFB_XC = D_INNER // P        # 16 xc feature blocks (full d_inner)
FF_SLICE = 1024              # ffn hidden slice per core
NB = DT_RANK + 2 * D_STATE   # 96

_GROUPS = [[0, 1, 4, 5], [2, 3, 6, 7]]


def _install_ntff_hook_shim(so_path="/opt/axon/libaxon_pjrt.so"):
    if "antenv.axon_hooks" in sys.modules:
        return
    try:
        lib = ctypes.CDLL(so_path)
    except OSError:
        return
    if not hasattr(lib, "axon_start_nrt_profile"):
        return
    lib.axon_start_nrt_profile.argtypes = [ctypes.POINTER(ctypes.c_int64), ctypes.c_size_t]
    lib.axon_start_nrt_profile.restype = ctypes.c_int64
    lib.axon_stop_nrt_profile.argtypes = [ctypes.c_char_p]
    lib.axon_stop_nrt_profile.restype = ctypes.c_int64

    @contextlib.contextmanager
    def _hook(output_dir, device_ids):
        import jax
        jax.devices()
        if device_ids:
            ids = (ctypes.c_int64 * len(device_ids))(*device_ids)
            rc = lib.axon_start_nrt_profile(ids, len(device_ids))
        else:
            rc = lib.axon_start_nrt_profile(None, 0)
        if rc != 0:
            raise RuntimeError(f"axon_start_nrt_profile rc={rc}")
        try:
            yield
        finally:
            n = lib.axon_stop_nrt_profile(str(output_dir).encode())
            print(f"profile: {n} file(s) written to {output_dir}", file=sys.stderr)

    mod = types.ModuleType("antenv.axon_hooks")
    mod.get_axon_ntff_profile_hook = lambda: _hook
    mod.set_axon_ntff_profile_hook = lambda h: None
    sys.modules["antenv.axon_hooks"] = mod


def _build_nc():
    from concourse import bacc, tile, mybir

    f32 = mybir.dt.float32
    f32r = mybir.dt.float32r
    bf16 = mybir.dt.bfloat16
    Alu = mybir.AluOpType
    Act = mybir.ActivationFunctionType

    def r(ap):
        return ap.bitcast(f32r)

    nc = bacc.Bacc("TRN2", target_bir_lowering=False, debug=False, num_devices=8)

    def din(name, shape, dt=None):
        return nc.dram_tensor(name, list(shape), dt or f32, kind="ExternalInput").ap()

    xT = din("xT", (D_MODEL, L), f32r)
    w_in_t = din("w_in_t", (NJ + NFB_XC, KD, P, P), f32r)          # z-half blocks, then xc
    conv_diag = din("conv_diag", (NFB_XC, D_CONV, P, P), f32r)
    convb_cols = din("convb_cols", (P, NFB_XC))
    xpw_t = din("xpw_t", (NFB_XC, P, P), f32r)  # cols: dt64|B16|pad|C16|pad
    dtw_t = din("dtw_t", (NJ, DT_RANK, P), f32r)
    dtb_cols = din("dtb_cols", (P, NJ))
    A_cols = din("A_cols", (P, NJ * D_STATE))
    D_colsT = din("D_colsT", (P, NJ))
    outw_t = din("outw_t", (KD, NJ, P, P), f32r)                   # [k(d_in), m(dm)]
    lng_cols = din("lng_cols", (P, KD))
    lnb_cols = din("lnb_cols", (P, KD))
    w1_t = din("w1_t", (KD, NJ, P, P), f32r)                       # [k(dm), m(h)]
    b1_cols = din("b1_cols", (P, NJ))
    w2_t = din("w2_t", (NJ, KD, P, P), f32r)                       # [k(h), m(dm)]
    b2_cols = din("b2_cols", (P, KD))
    consts_r = din("consts_r", (P, 4), f32r)  # col0=1/1024, cols1..3=0
    ident_r = din("ident_r", (P, P), f32r)

    out_m = nc.dram_tensor("out_m", [D_MODEL, L], f32, kind="ExternalOutput").ap()

    es = contextlib.ExitStack()

    with tile.TileContext(nc) as tc:
        with contextlib.ExitStack() as stk:
            cpool = stk.enter_context(tc.tile_pool(name="cpool", bufs=1))
            psum = stk.enter_context(tc.tile_pool(name="psum", bufs=4, space="PSUM"))
            dram = stk.enter_context(tc.tile_pool(name="dram", bufs=1, space="DRAM"))

            def cload(src, shape, tag):
                t = cpool.tile(list(shape), f32, tag=tag, name=tag)
                nc.sync.dma_start(t[:], src)
                return t

            A_sb = cload(A_cols[:], (P, NJ * D_STATE), "A_sb")
            dtb_sb = cload(dtb_cols[:], (P, NJ), "dtb_sb")
            D_sb = cload(D_colsT[:], (P, NJ), "D_sb")
            convb_sb = cload(convb_cols[:], (P, NFB_XC), "convb_sb")
            lng_sb = cload(lng_cols[:], (P, KD), "lng_sb")
            lnb_sb = cload(lnb_cols[:], (P, KD), "lnb_sb")
            b1_sb = cload(b1_cols[:], (P, NJ), "b1_sb")
            b2_sb = cload(b2_cols[:], (P, KD), "b2_sb")
            ones_sb = cpool.tile([P, 1], f32r, tag="ones_sb", name="ones_sb")
            nc.sync.dma_start(ones_sb[:], consts_r[:, 0:1])
            ident_sb = cpool.tile([P, P], f32r, tag="ident_sb", name="ident_sb")
            nc.sync.dma_start(ident_sb[:], ident_r[:])

            bcB_dram = dram.tile([D_STATE, L], bf16, name="bcB_dram")
            bcC_dram = dram.tile([D_STATE, L], f32, name="bcC_dram")
            stat_dram = dram.tile([2, L], f32, name="stat_dram")
            ag_in = dram.tile([D_MODEL, L], f32, name="ag_in")
            ag_out = dram.tile([4 * D_MODEL, L], f32, name="ag_out")
            ar_in = dram.tile([D_MODEL, L], f32, name="ar_in")
            ar_out = dram.tile([D_MODEL, L], f32, name="ar_out")

            def mm_accum(ps, lw_list, rhs_of_k, n_k):
                for k in range(n_k):
                    nc.tensor.matmul(ps[:], lw_list[k][:], rhs_of_k(k),
                                     start=(k == 0), stop=(k == n_k - 1))

            # =========== P1-P4 region: sz lives until gating ===========
            with tc.tile_pool(name="sz_pool", bufs=1) as sz_pool:
                sz = [sz_pool.tile([P, L], f32, tag=f"sz{j}", name=f"sz{j}")
                      for j in range(NJ)]
                dt_sb = sz_pool.tile([DT_RANK, L], f32r, tag="dt_sb", name="dt_sb")

                def dt_proj_delta(j, delta_t, pool):
                    # softplus(x + b) = Ln(1 + Exp(x + b)); inputs here are
                    # well below 0 so Exp cannot overflow.
                    lw = pool.tile([DT_RANK, P], f32r, tag="dtw", name=f"dtw{j}", bufs=2)
                    nc.sync.dma_start(lw[:], dtw_t[j])
                    for t in range(NT):
                        ps = psum.tile([P, TCH], f32, tag="ps", name=f"dtp{j}_{t}")
                        nc.tensor.matmul(ps[:], lw[:],
                                         dt_sb[:, t * TCH:(t + 1) * TCH],
                                         start=True, stop=True)
                        spt = pool.tile([P, TCH], f32, tag="spt", name=f"spt{j}_{t}",
                                        bufs=2)
                        nc.scalar.activation(spt[:], ps[:], Act.Exp,
                                             bias=dtb_sb[:, j:j + 1])
                        nc.scalar.activation(delta_t[:, t * TCH:(t + 1) * TCH], spt[:],
                                             Act.Ln, bias=1.0)

                # ---------------- P1..P3: need xc blocks ----------------
                with tc.tile_pool(name="xc_pool", bufs=1) as xc_pool:
                    xcs = [xc_pool.tile([P, L], f32r, tag=f"xcs{j}", name=f"xcs{j}")
                           for j in range(NFB_XC)]

                    # P1: in_proj + conv + silu
                    with tc.tile_pool(name="xt_pool", bufs=1) as xt_pool, \
                         tc.tile_pool(name="p1t", bufs=1) as p1t:
                        xts = []
                        for k in range(KD):
                            xt_k = xt_pool.tile([P, L], f32r, tag=f"xt{k}", name=f"xt{k}")
                            nc.sync.dma_start(xt_k[:], xT[k * P:(k + 1) * P, :])
                            xts.append(xt_k)

                        def in_proj_block(fb):
                            lws = []
                            for k in range(KD):
                                lw = p1t.tile([P, P], f32r, tag=f"lw{k}",
                                              name=f"lw{fb}_{k}", bufs=2)
                                nc.sync.dma_start(lw[:], w_in_t[fb, k])
                                lws.append(lw)
                            pss = []
                            for t in range(NT):
                                ps = psum.tile([P, TCH], f32, tag="ps",
                                               name=f"inp{fb}_{t}")
                                mm_accum(ps, lws,
                                         lambda k: xts[k][:, t * TCH:(t + 1) * TCH], KD)
                                pss.append(ps)
                            return pss

                        for j in range(NJ):  # z half
                            for t, ps in enumerate(in_proj_block(j)):
                                nc.scalar.activation(sz[j][:, t * TCH:(t + 1) * TCH],
                                                     ps[:], Act.Silu)

                        for j in range(NFB_XC):  # xc blocks + conv
                            xcp = p1t.tile([P, L + D_CONV - 1], f32r, tag="xcp",
                                           name=f"xcp{j}", bufs=2)
                            nc.sync.dma_start(xcp[:, 0:D_CONV - 1], consts_r[:, 1:D_CONV])
                            for t, ps in enumerate(in_proj_block(NJ + j)):
                                nc.scalar.copy(
                                    xcp[:, D_CONV - 1 + t * TCH:D_CONV - 1 + (t + 1) * TCH],
                                    ps[:])
                            cds = []
                            for i in range(D_CONV):
                                cd = p1t.tile([P, P], f32r, tag=f"cd{i}",
                                              name=f"cd{j}_{i}", bufs=2)
                                nc.sync.dma_start(cd[:], conv_diag[j, i])
                                cds.append(cd)
                            for t in range(NT):
                                cps = psum.tile([P, TCH], f32, tag="ps",
                                                name=f"cps{j}_{t}")
                                mm_accum(cps, cds,
                                         lambda i: xcp[:, t * TCH + i:t * TCH + i + TCH],
                                         D_CONV)
                                nc.scalar.activation(xcs[j][:, t * TCH:(t + 1) * TCH],
                                                     cps[:], Act.Silu,
                                                     bias=convb_sb[:, j:j + 1])

                    # right-side pools for wv/g0 (live P2..P4)
                    wvg_ctx = contextlib.ExitStack()
                    wv_pool = wvg_ctx.enter_context(
                        tc.tile_pool(name="wv_pool", bufs=1, side="right"))
                    wvs = [wv_pool.tile([P, L], bf16, tag=f"wv{j}", name=f"wv{j}")
                           for j in range(NJ)]
                    g0_pool = wvg_ctx.enter_context(
                        tc.tile_pool(name="g0_pool", bufs=1, side="right"))
                    g0s = [g0_pool.tile([P, L], f32, tag=f"g0{j}", name=f"g0{j}")
                           for j in range(NJ)]

                    # P2: x_proj; P3: wv/g0
                    with tc.tile_pool(name="p2t", bufs=1) as p2t:
                        for t in range(NT):
                            ps = psum.tile([P, TCH], f32, tag="ps", name=f"xproj{t}")
                            for k in range(NFB_XC):
                                lw = p2t.tile([P, P], f32r, tag="xpw",
                                              name=f"xpw{t}_{k}", bufs=2)
                                nc.sync.dma_start(lw[:], xpw_t[k])
                                nc.tensor.matmul(ps[:], lw[:],
                                                 xcs[k][:, t * TCH:(t + 1) * TCH],
                                                 start=(k == 0), stop=(k == NFB_XC - 1))
                            nc.scalar.copy(dt_sb[:, t * TCH:(t + 1) * TCH],
                                           ps[0:DT_RANK, :])
                            bcB_sb = p2t.tile([D_STATE, TCH], bf16, tag="bcB_sb",
                                              name=f"bcB_sb{t}", bufs=2)
                            nc.scalar.copy(bcB_sb[:], ps[64:80, :])
                            nc.sync.dma_start(bcB_dram[:, t * TCH:(t + 1) * TCH], bcB_sb[:])
                            bcC_sb = p2t.tile([D_STATE, TCH], f32, tag="bcC_sb",
                                              name=f"bcC_sb{t}", bufs=2)
                            nc.scalar.copy(bcC_sb[:], ps[96:112, :])
                            nc.sync.dma_start(bcC_dram[:, t * TCH:(t + 1) * TCH], bcC_sb[:])

                        for j in range(NJ):
                            dtmp = p2t.tile([P, L], f32, tag="dtmp", name=f"dtmp{j}",
                                            bufs=2)
                            dt_proj_delta(j, dtmp, p2t)
                            nc.vector.tensor_tensor(wvs[j][:], dtmp[:], xcs[j][:],
                                                    Alu.mult)
                            t1 = p2t.tile([P, L], f32, tag="g0tmp", name=f"g0tmp{j}",
                                          bufs=2)
                            nc.vector.tensor_scalar_mul(t1[:], xcs[j][:], D_sb[:, j:j + 1])
                            nc.vector.tensor_tensor(g0s[j][:], t1[:], sz[j][:], Alu.mult)

                # ---------------- P4: scan + gating ----------------
                yg_ctx = contextlib.ExitStack()
                yg_pool = yg_ctx.enter_context(tc.tile_pool(name="yg_pool", bufs=1))
                ygs = [yg_pool.tile([P, L], f32r, tag=f"yg{j}", name=f"yg{j}")
                       for j in range(NJ)]
                with tc.tile_pool(name="spool", bufs=1) as spool, \
                     tc.tile_pool(name="tpool", bufs=1) as tpool, \
                     tc.tile_pool(name="pscan", bufs=1, space="PSUM") as pscan:
                    for hb in range(4):
                        js = list(range(hb * 2, hb * 2 + 2))
                        deltas = {}
                        yps = {}
                        for j in js:
                            dj = spool.tile([P, L], f32, tag=f"delta{j % 2}",
                                            name=f"delta{j}")
                            dt_proj_delta(j, dj, spool)
                            deltas[j] = dj
                            yps[j] = pscan.tile([P, L], f32, tag=f"yps{j % 2}",
                                                name=f"yps{j}")
                        for n in range(D_STATE):
                            Bbc = tpool.tile([P, L], bf16, tag="Bbc",
                                             name=f"Bbc{hb}_{n}", bufs=2)
                            nc.sync.dma_start(
                                Bbc[:],
                                bcB_dram[n:n + 1, :].partition_broadcast(P).squeeze(1))
                            Cbc = tpool.tile([P, L], f32, tag="Cbc",
                                             name=f"Cbc{hb}_{n}", bufs=2)
                            nc.sync.dma_start(
                                Cbc[:],
                                bcC_dram[n:n + 1, :].partition_broadcast(P).squeeze(1))
                            for j in js:
                                a_t = tpool.tile([P, L], f32, tag="a_t",
                                                 name=f"a{j}_{n}", bufs=2)
                                nc.scalar.activation(
                                    a_t[:], deltas[j][:], Act.Exp,
                                    scale=A_sb[:, j * D_STATE + n:j * D_STATE + n + 1])
                                b_t = tpool.tile([P, L], bf16, tag="b_t",
                                                 name=f"b{j}_{n}", bufs=1)
                                nc.vector.tensor_tensor(b_t[:], wvs[j][:], Bbc[:],
                                                        Alu.mult)
                                h_t = tpool.tile([P, L], f32, tag="h_t",
                                                 name=f"h{j}_{n}", bufs=1)
                                nc.vector.tensor_tensor_scan(h_t[:], a_t[:], b_t[:],
                                                             0.0, Alu.mult, Alu.add)
                                prod = tpool.tile([P, L], f32r, tag="prod",
                                                  name=f"p{j}_{n}", bufs=3)
                                nc.vector.tensor_tensor(prod[:], h_t[:], Cbc[:],
                                                        Alu.mult)
                                for t in range(NT):
                                    sl = slice(t * TCH, (t + 1) * TCH)
                                    nc.tensor.matmul(yps[j][:, sl], ident_sb[:],
                                                     prod[:, sl],
                                                     start=(n == 0),
                                                     stop=(n == D_STATE - 1))
                        for j in js:
                            ygt = tpool.tile([P, L], f32, tag="ygt",
                                             name=f"ygt{j}", bufs=2)
                            nc.vector.tensor_tensor(ygt[:], yps[j][:], sz[j][:],
                                                    Alu.mult)
                            nc.vector.tensor_tensor(ygs[j][:], ygt[:], g0s[j][:],
                                                    Alu.add)
                wvg_ctx.close()  # wv + g0 (right side, LIFO: g0 then wv)

                # =========== P5: out_proj partials ===========
                with tc.tile_pool(name="p5t", bufs=1) as p5t:
                    for m in range(NJ):
                        lws = []
                        for k in range(KD):
                            lw = p5t.tile([P, P], f32r, tag=f"lw{k}",
                                          name=f"ow{m}_{k}", bufs=2)
                            nc.sync.dma_start(lw[:], outw_t[k, m])
                            lws.append(lw)
                        msb = p5t.tile([P, L], f32, tag="msb", name=f"msb{m}", bufs=2)
                        for t in range(NT):
                            ps = psum.tile([P, TCH], f32, tag="ps", name=f"op{m}_{t}")
                            mm_accum(ps, lws,
                                     lambda k: ygs[k][:, t * TCH:(t + 1) * TCH], KD)
                            nc.scalar.copy(msb[:, t * TCH:(t + 1) * TCH], ps[:])
                        nc.sync.dma_start(ag_in[m * P:(m + 1) * P, :], msb[:])
                yg_ctx.close()

            nc.gpsimd.collective_compute("AllGather", Alu.bypass,
                                         replica_groups=_GROUPS,
                                         ins=[ag_in[:]], outs=[ag_out[:]])

            # =========== P6: mo + LN ===========
            with contextlib.ExitStack() as stk2:
                mo_pool = stk2.enter_context(tc.tile_pool(name="mo_pool", bufs=1))
                mos = [mo_pool.tile([P, L], f32r, tag=f"mo{j}", name=f"mo{j}")
                       for j in range(KD)]
                xn_pool = stk2.enter_context(tc.tile_pool(name="xn_pool", bufs=1))
                xns = [xn_pool.tile([P, L], f32r, tag=f"xn{j}", name=f"xn{j}")
                       for j in range(KD)]

                with tc.tile_pool(name="p6t", bufs=1) as p6t, \
                     tc.tile_pool(name="pln", bufs=1, space="PSUM") as pln:
                    mu_ps = pln.tile([1, L], f32, tag="mu_ps", name="mu_ps", bufs=1)
                    e2_ps = pln.tile([1, L], f32, tag="e2_ps", name="e2_ps", bufs=1)
                    for j in range(KD):
                        parts = []
                        for q in range(4):
                            pt = p6t.tile([P, L], f32, tag="agp", name=f"agp{j}_{q}",
                                          bufs=4)
                            nc.sync.dma_start(
                                pt[:],
                                ag_out[q * D_MODEL + j * P:q * D_MODEL + (j + 1) * P, :])
                            parts.append(pt)
                        a01 = p6t.tile([P, L], f32, tag="a01", name=f"a01_{j}", bufs=2)
                        nc.vector.tensor_tensor(a01[:], parts[0][:], parts[1][:],
                                                Alu.add)
                        a23 = p6t.tile([P, L], f32, tag="a23", name=f"a23_{j}", bufs=2)
                        nc.vector.tensor_tensor(a23[:], parts[2][:], parts[3][:],
                                                Alu.add)
                        nc.vector.tensor_tensor(mos[j][:], a01[:], a23[:, ::-1],
                                                Alu.add)
                        sq = p6t.tile([P, L], f32r, tag="sq", name=f"sq{j}", bufs=2)
                        nc.scalar.activation(sq[:], mos[j][:], Act.Square)
                        for t in range(NT):
                            sl = slice(t * TCH, (t + 1) * TCH)
                            nc.tensor.matmul(mu_ps[:, sl], ones_sb[:],
                                             mos[j][:, sl],
                                             start=(j == 0), stop=(j == KD - 1))
                            nc.tensor.matmul(e2_ps[:, sl], ones_sb[:],
                                             sq[:, sl],
                                             start=(j == 0), stop=(j == KD - 1))

                    mean_sb = p6t.tile([1, L], f32, tag="mean_sb", name="mean_sb",
                                       bufs=1)
                    nc.scalar.copy(mean_sb[:], mu_ps[:])
                    m2 = p6t.tile([1, L], f32, tag="m2", name="m2", bufs=1)
                    nc.vector.tensor_tensor(m2[:], mean_sb[:], mean_sb[:], Alu.mult)
                    var_t = p6t.tile([1, L], f32, tag="var_t", name="var_t", bufs=1)
                    nc.vector.tensor_tensor(var_t[:], e2_ps[:], m2[:], Alu.subtract)
                    eps_sb = p6t.tile([1, 1], f32, tag="eps_sb", name="eps_sb", bufs=1)
                    nc.vector.memset(eps_sb[:], 1e-5)
                    std_t = p6t.tile([1, L], f32, tag="std_t", name="std_t", bufs=1)
                    nc.scalar.activation(std_t[:], var_t[:], Act.Sqrt, bias=eps_sb[:])
                    rstd_sb = p6t.tile([1, L], f32, tag="rstd_sb", name="rstd_sb",
                                       bufs=1)
                    nc.vector.reciprocal(rstd_sb[:], std_t[:])
                    nc.sync.dma_start(stat_dram[0:1, :], mean_sb[:])
                    nc.sync.dma_start(stat_dram[1:2, :], rstd_sb[:])
                    mean_bc = p6t.tile([P, L], f32, tag="mean_bc", name="mean_bc",
                                       bufs=1)
                    nc.sync.dma_start(
                        mean_bc[:],
                        stat_dram[0:1, :].partition_broadcast(P).squeeze(1))
                    rstd_bc = p6t.tile([P, L], f32, tag="rstd_bc", name="rstd_bc",
                                       bufs=1)
                    nc.sync.dma_start(
                        rstd_bc[:],
                        stat_dram[1:2, :].partition_broadcast(P).squeeze(1))

                    for j in range(KD):
                        t1 = p6t.tile([P, L], f32, tag="lnt", name=f"lnt{j}", bufs=2)
                        nc.vector.tensor_tensor(t1[:], mos[j][:], mean_bc[:],
                                                Alu.subtract)
                        nc.vector.tensor_tensor(t1[:], t1[:], rstd_bc[:], Alu.mult)
                        nc.vector.tensor_scalar(xns[j][:], t1[:], lng_sb[:, j:j + 1],
                                                lnb_sb[:, j:j + 1], Alu.mult, Alu.add)

                # =========== P7: FFN ===========
                with tc.tile_pool(name="ffh_pool", bufs=1) as ffh_pool, \
                     tc.tile_pool(name="p7t", bufs=1) as p7t:
                    ffhs = [ffh_pool.tile([P, L], f32r, tag=f"ffh{m}", name=f"ffh{m}")
                            for m in range(NJ)]
                    for m in range(NJ):
                        lws = []
                        for k in range(KD):
                            lw = p7t.tile([P, P], f32r, tag=f"lw{k}", name=f"w1_{m}_{k}",
                                          bufs=2)
                            nc.sync.dma_start(lw[:], w1_t[k, m])
                            lws.append(lw)
                        for t in range(NT):
                            ps = psum.tile([P, TCH], f32, tag="ps", name=f"f1{m}_{t}")
                            mm_accum(ps, lws,
                                     lambda k: xns[k][:, t * TCH:(t + 1) * TCH], KD)
                            nc.scalar.activation(ffhs[m][:, t * TCH:(t + 1) * TCH],
                                                 ps[:], Act.Gelu,
                                                 bias=b1_sb[:, m:m + 1])

                    for m in range(KD):
                        lws = []
                        for k in range(NJ):
                            lw = p7t.tile([P, P], f32r, tag=f"lw{k}", name=f"w2_{m}_{k}",
                                          bufs=2)
                            nc.sync.dma_start(lw[:], w2_t[k, m])
                            lws.append(lw)
                        msb = p7t.tile([P, L], f32, tag="msb", name=f"f2sb{m}", bufs=2)
                        for t in range(NT):
                            ps = psum.tile([P, TCH], f32, tag="ps", name=f"f2{m}_{t}")
                            mm_accum(ps, lws,
                                     lambda k: ffhs[k][:, t * TCH:(t + 1) * TCH], NJ)
                            nc.scalar.copy(msb[:, t * TCH:(t + 1) * TCH], ps[:])
                        nc.sync.dma_start(ar_in[m * P:(m + 1) * P, :], msb[:])

            nc.gpsimd.collective_compute("AllReduce", Alu.add, replica_groups=_GROUPS,
                                         ins=[ar_in[:]], outs=[ar_out[:]])

            with tc.tile_pool(name="p8t", bufs=1) as p8t:
                for j in range(KD):
                    fin = p8t.tile([P, L], f32, tag="fin", name=f"fin{j}", bufs=2)
                    nc.sync.dma_start(fin[:], ar_out[j * P:(j + 1) * P, :])
                    fob = p8t.tile([P, L], f32, tag="fob", name=f"fob{j}", bufs=2)
                    nc.vector.tensor_scalar_add(fob[:], fin[:], b2_sb[:, j:j + 1])
                    nc.sync.dma_start(out_m[j * P:(j + 1) * P, :], fob[:])

    nc.compile()
    return nc


def _prep_inputs(inputs):
    """Per-core input dicts. Core c: sequence s=c//2 (s>=2 => time-flipped x),
    d_inner half = c%2. The own half of d_inner is permuted FIRST in every
    d_inner-ordered tensor, so the device kernel is identical on all cores."""
    x = np.asarray(inputs["x"], dtype=np.float32)
    in_proj_w = np.asarray(inputs["in_proj_w"], dtype=np.float32)
    conv_w = np.asarray(inputs["conv_w"], dtype=np.float32)
    conv_b = np.asarray(inputs["conv_b"], dtype=np.float32)
    x_proj_w = np.asarray(inputs["x_proj_w"], dtype=np.float32)
    dt_proj_w = np.asarray(inputs["dt_proj_w"], dtype=np.float32)
    dt_proj_b = np.asarray(inputs["dt_proj_b"], dtype=np.float32)
    A = -np.exp(np.asarray(inputs["A_log"], dtype=np.float32))
    Dp = np.asarray(inputs["D"], dtype=np.float32)
    out_proj_w = np.asarray(inputs["out_proj_w"], dtype=np.float32)
    ln_g = np.asarray(inputs["ln_g"], dtype=np.float32)
    ln_b = np.asarray(inputs["ln_b"], dtype=np.float32)
    ff_w1 = np.asarray(inputs["ff_w1"], dtype=np.float32)
    ff_b1 = np.asarray(inputs["ff_b1"], dtype=np.float32)
    ff_w2 = np.asarray(inputs["ff_w2"], dtype=np.float32)
    ff_b2 = np.asarray(inputs["ff_b2"], dtype=np.float32)

    def cols(v):  # (N,) -> (P, N//P) per-partition column layout
        return np.ascontiguousarray(v.reshape(-1, P).T)

    def tile_w(w, KP, MP):  # (K, M) -> (K//KP, M//MP, KP, MP)
        K, M = w.shape
        return np.ascontiguousarray(
            w.reshape(K // KP, KP, M // MP, MP).transpose(0, 2, 1, 3))

    in_maps = []
    for c in range(8):
        s, half = c // 2, c % 2
        xb = x[s] if s < 2 else x[s - 2][::-1]
        perm = np.arange(D_INNER).reshape(2, HALF)
        perm = np.concatenate([perm[half], perm[1 - half]])
        own = perm[:HALF]

        wz = in_proj_w[:, D_INNER + own]                      # (1024, 1024)
        wxc = in_proj_w[:, perm]                              # (1024, 2048)
        w_in = np.concatenate([wz, wxc], axis=1)              # (1024, 3072)
        w_in_t = np.ascontiguousarray(tile_w(w_in, P, P).transpose(1, 0, 2, 3))

        cw = conv_w[perm]
        conv_diag = np.zeros((NFB_XC, D_CONV, P, P), np.float32)
        idx = np.arange(P)
        for j in range(NFB_XC):
            for i in range(D_CONV):
                conv_diag[j, i, idx, idx] = cw[j * P:(j + 1) * P, i]

        g = (c & 1) + 2 * (c >> 2)
        hsl = slice(g * FF_SLICE, (g + 1) * FF_SLICE)

        in_maps.append({
            "xT": np.ascontiguousarray(xb.T),
            "w_in_t": w_in_t,
            "conv_diag": conv_diag,
            "convb_cols": cols(conv_b[perm]),
            "xpw_t": np.ascontiguousarray(
                np.concatenate([
                    x_proj_w[perm][:, :DT_RANK + D_STATE],
                    np.zeros((D_INNER, D_STATE), np.float32),
                    x_proj_w[perm][:, DT_RANK + D_STATE:],
                    np.zeros((D_INNER, D_STATE), np.float32),
                ], axis=1).reshape(NFB_XC, P, P)),
            "dtw_t": np.ascontiguousarray(
                dt_proj_w[:, own].reshape(DT_RANK, NJ, P).transpose(1, 0, 2)),
            "dtb_cols": cols(dt_proj_b[own]),
            "A_cols": np.ascontiguousarray(
                A[own].reshape(NJ, P, D_STATE).transpose(1, 0, 2).reshape(P, NJ * D_STATE)),
            "D_colsT": cols(Dp[own]),
            "outw_t": tile_w(out_proj_w[own], P, P),
            "lng_cols": cols(ln_g),
            "lnb_cols": cols(ln_b),
            "w1_t": tile_w(ff_w1[:, hsl], P, P),
            "b1_cols": cols(ff_b1[hsl]),
            "w2_t": tile_w(ff_w2[hsl], P, P),
            "b2_cols": cols(ff_b2),
            "ident_r": np.eye(P, dtype=np.float32),
            "consts_r": np.concatenate(
                [np.full((P, 1), 1.0 / D_MODEL, np.float32),
                 np.zeros((P, 3), np.float32)], axis=1),
        })
    return in_maps


_NC_CACHE = {}


def _get_nc():
    if "nc" not in _NC_CACHE:
        _NC_CACHE["nc"] = _build_nc()
    return _NC_CACHE["nc"]


def run(inputs, trace=False):
    _install_ntff_hook_shim()
    from concourse import bass_utils
    nc = _get_nc()
    in_maps = _prep_inputs(inputs)
    res = bass_utils.run_bass_kernel_spmd(nc, in_maps, core_ids=list(range(8)),
                                          trace=trace)
    out = np.stack([
        np.ascontiguousarray(res.results[0]["out_m"].T),
        np.ascontiguousarray(res.results[2]["out_m"].T),
    ]).astype(np.float32)
    return out, res


def kernel(**inputs):
    out, _ = run(inputs, trace=False)
    return out


# revision 15
# speedup vs baseline: 1.2118x; 1.0375x over previous
"""BiMamba (bidirectional Mamba block + LN + FFN) Trainium2 Bass kernel.

Sharding (8 cores): 4 scan-sequences (fwd/bwd x batch, bwd fed host-flipped x)
x 2 halves of d_inner. Device layout is feature-on-partitions /
time-on-free throughout; the host transposes x on the way in and the output
on the way out. Cross-core combines (out_proj partial sums + direction
merge, ff2 partial sums) use AllGather/AllReduce over quads
[0,1,4,5] / [2,3,6,7].
"""
import sys, os, types, contextlib, ctypes

sys.path.insert(0, "/opt/trn_rl_repo")
import numpy as np

D_MODEL = 1024
D_STATE = 16
D_CONV = 4
D_INNER = 2048
DT_RANK = 64
L = 1024
HALF = D_INNER // 2          # 1024 d_inner per core
P = 128
NJ = HALF // P               # 8 d-blocks per core half
TCH = 512                    # matmul t-chunk
NT = L // TCH
KD = D_MODEL // P            # 8 k-chunks over d_model
NFB_XC = D_INNER // P        # 16 xc feature blocks (full d_inner)
FF_SLICE = 1024              # ffn hidden slice per core
NB = DT_RANK + 2 * D_STATE   # 96

_GROUPS = [[0, 1, 4, 5], [2, 3, 6, 7]]


def _install_ntff_hook_shim(so_path="/opt/axon/libaxon_pjrt.so"):
    if "antenv.axon_hooks" in sys.modules:
        return
    try:
        lib = ctypes.CDLL(so_path)
    except OSError:
        return
    if not hasattr(lib, "axon_start_nrt_profile"):
        return
    lib.axon_start_nrt_profile.argtypes = [ctypes.POINTER(ctypes.c_int64), ctypes.c_size_t]
    lib.axon_start_nrt_profile.restype = ctypes.c_int64
    lib.axon_stop_nrt_profile.argtypes = [ctypes.c_char_p]
    lib.axon_stop_nrt_profile.restype = ctypes.c_int64

    @contextlib.contextmanager
    def _hook(output_dir, device_ids):
        import jax
        jax.devices()
        if device_ids:
            ids = (ctypes.c_int64 * len(device_ids))(*device_ids)
            rc = lib.axon_start_nrt_profile(ids, len(device_ids))
        else:
            rc = lib.axon_start_nrt_profile(None, 0)
        if rc != 0:
            raise RuntimeError(f"axon_start_nrt_profile rc={rc}")
        try:
            yield
        finally:
            n = lib.axon_stop_nrt_profile(str(output_dir).encode())
            print(f"profile: {n} file(s) written to {output_dir}", file=sys.stderr)

    mod = types.ModuleType("antenv.axon_hooks")
    mod.get_axon_ntff_profile_hook = lambda: _hook
    mod.set_axon_ntff_profile_hook = lambda h: None
    sys.modules["antenv.axon_hooks"] = mod


def _build_nc():
    from concourse import bacc, tile, mybir

    f32 = mybir.dt.float32
    f32r = mybir.dt.float32r
    bf16 = mybir.dt.bfloat16
    Alu = mybir.AluOpType
    Act = mybir.ActivationFunctionType

    def r(ap):
        return ap.bitcast(f32r)

    nc = bacc.Bacc("TRN2", target_bir_lowering=False, debug=False, num_devices=8)

    def din(name, shape, dt=None):
        return nc.dram_tensor(name, list(shape), dt or f32, kind="ExternalInput").ap()

    xT = din("xT", (D_MODEL, L), f32r)
    w_in_t = din("w_in_t", (NJ + NFB_XC, KD, P, P), f32r)          # z-half blocks, then xc
    convw_cols = din("convw_cols", (P, NFB_XC * D_CONV))
    convb_cols = din("convb_cols", (P, NFB_XC))
    xpw_t = din("xpw_t", (NFB_XC, P, P), f32r)  # cols: dt64|B16|pad|C16|pad
    dtw_t = din("dtw_t", (NJ, DT_RANK, P), f32r)
    dtb_cols = din("dtb_cols", (P, NJ))
    A_cols = din("A_cols", (P, NJ * D_STATE))
    D_colsT = din("D_colsT", (P, NJ))
    outw_t = din("outw_t", (KD, NJ, P, P), f32r)                   # [k(d_in), m(dm)]
    lng_cols = din("lng_cols", (P, KD))
    lnb_cols = din("lnb_cols", (P, KD))
    w1_t = din("w1_t", (KD, NJ, P, P), f32r)                       # [k(dm), m(h)]
    b1_cols = din("b1_cols", (P, NJ))
    w2_t = din("w2_t", (NJ, KD, P, P), f32r)                       # [k(h), m(dm)]
    b2_cols = din("b2_cols", (P, KD))
    consts_r = din("consts_r", (P, 4), f32r)  # col0=1/1024, cols1..3=0
    ident_r = din("ident_r", (P, P), f32r)

    out_m = nc.dram_tensor("out_m", [D_MODEL, L], f32, kind="ExternalOutput").ap()

    es = contextlib.ExitStack()

    with tile.TileContext(nc) as tc:
        with contextlib.ExitStack() as stk:
            cpool = stk.enter_context(tc.tile_pool(name="cpool", bufs=1))
            psum = stk.enter_context(tc.tile_pool(name="psum", bufs=4, space="PSUM"))
            dram = stk.enter_context(tc.tile_pool(name="dram", bufs=1, space="DRAM"))

            def cload(src, shape, tag):
                t = cpool.tile(list(shape), f32, tag=tag, name=tag)
                nc.sync.dma_start(t[:], src)
                return t

            A_sb = cload(A_cols[:], (P, NJ * D_STATE), "A_sb")
            dtb_sb = cload(dtb_cols[:], (P, NJ), "dtb_sb")
            D_sb = cload(D_colsT[:], (P, NJ), "D_sb")
            convb_sb = cload(convb_cols[:], (P, NFB_XC), "convb_sb")
            convw_sb = cload(convw_cols[:], (P, NFB_XC * D_CONV), "convw_sb")
            lng_sb = cload(lng_cols[:], (P, KD), "lng_sb")
            lnb_sb = cload(lnb_cols[:], (P, KD), "lnb_sb")
            b1_sb = cload(b1_cols[:], (P, NJ), "b1_sb")
            b2_sb = cload(b2_cols[:], (P, KD), "b2_sb")
            ones_sb = cpool.tile([P, 1], f32r, tag="ones_sb", name="ones_sb")
            nc.sync.dma_start(ones_sb[:], consts_r[:, 0:1])
            ident_sb = cpool.tile([P, P], f32r, tag="ident_sb", name="ident_sb")
            nc.sync.dma_start(ident_sb[:], ident_r[:])

            bcB_dram = dram.tile([D_STATE, L], bf16, name="bcB_dram")
            bcC_dram = dram.tile([D_STATE, L], f32, name="bcC_dram")
            stat_dram = dram.tile([2, L], f32, name="stat_dram")
            ag_in = dram.tile([D_MODEL, L], f32, name="ag_in")
            ag_out = dram.tile([4 * D_MODEL, L], f32, name="ag_out")
            ar_in = dram.tile([D_MODEL, L], f32, name="ar_in")
            ar_out = dram.tile([D_MODEL, L], f32, name="ar_out")

            def mm_accum(ps, lw_list, rhs_of_k, n_k):
                for k in range(n_k):
                    nc.tensor.matmul(ps[:], lw_list[k][:], rhs_of_k(k),
                                     start=(k == 0), stop=(k == n_k - 1))

            # =========== P1-P4 region: sz lives until gating ===========
            with tc.tile_pool(name="sz_pool", bufs=1) as sz_pool:
                sz = [sz_pool.tile([P, L], f32, tag=f"sz{j}", name=f"sz{j}")
                      for j in range(NJ)]
                dt_sb = sz_pool.tile([DT_RANK, L], f32r, tag="dt_sb", name="dt_sb")

                def dt_proj_delta(j, delta_t, pool):
                    # softplus(x + b) = Ln(1 + Exp(x + b)); inputs here are
                    # well below 0 so Exp cannot overflow.
                    lw = pool.tile([DT_RANK, P], f32r, tag="dtw", name=f"dtw{j}", bufs=2)
                    nc.sync.dma_start(lw[:], dtw_t[j])
                    for t in range(NT):
                        ps = psum.tile([P, TCH], f32, tag="ps", name=f"dtp{j}_{t}")
                        nc.tensor.matmul(ps[:], lw[:],
                                         dt_sb[:, t * TCH:(t + 1) * TCH],
                                         start=True, stop=True)
                        spt = pool.tile([P, TCH], f32, tag="spt", name=f"spt{j}_{t}",
                                        bufs=2)
                        nc.scalar.activation(spt[:], ps[:], Act.Exp,
                                             bias=dtb_sb[:, j:j + 1])
                        nc.scalar.activation(delta_t[:, t * TCH:(t + 1) * TCH], spt[:],
                                             Act.Ln, bias=1.0)

                # ---------------- P1..P3: need xc blocks ----------------
                with tc.tile_pool(name="xc_pool", bufs=1) as xc_pool:
                    xcs = [xc_pool.tile([P, L], f32r, tag=f"xcs{j}", name=f"xcs{j}")
                           for j in range(NFB_XC)]

                    # P1: in_proj + conv + silu
                    with tc.tile_pool(name="xt_pool", bufs=1) as xt_pool, \
                         tc.tile_pool(name="p1t", bufs=1) as p1t:
                        xts = []
                        for k in range(KD):
                            xt_k = xt_pool.tile([P, L], f32r, tag=f"xt{k}", name=f"xt{k}")
                            nc.sync.dma_start(xt_k[:], xT[k * P:(k + 1) * P, :])
                            xts.append(xt_k)

                        def in_proj_block(fb):
                            lws = []
                            for k in range(KD):
                                lw = p1t.tile([P, P], f32r, tag=f"lw{k}",
                                              name=f"lw{fb}_{k}", bufs=2)
                                nc.sync.dma_start(lw[:], w_in_t[fb, k])
                                lws.append(lw)
                            pss = []
                            for t in range(NT):
                                ps = psum.tile([P, TCH], f32, tag="ps",
                                               name=f"inp{fb}_{t}")
                                mm_accum(ps, lws,
                                         lambda k: xts[k][:, t * TCH:(t + 1) * TCH], KD)
                                pss.append(ps)
                            return pss

                        for j in range(NJ):  # z half
                            for t, ps in enumerate(in_proj_block(j)):
                                nc.scalar.activation(sz[j][:, t * TCH:(t + 1) * TCH],
                                                     ps[:], Act.Silu)

                        for j in range(NFB_XC):  # xc blocks + conv (DVE taps)
                            xcp = p1t.tile([P, L + D_CONV - 1], f32, tag="xcp",
                                           name=f"xcp{j}", bufs=2)
                            nc.sync.dma_start(xcp[:, 0:D_CONV - 1],
                                              consts_r[:, 1:D_CONV].bitcast(f32))
                            for t, ps in enumerate(in_proj_block(NJ + j)):
                                nc.scalar.copy(
                                    xcp[:, D_CONV - 1 + t * TCH:D_CONV - 1 + (t + 1) * TCH],
                                    ps[:])
                            cacc = p1t.tile([P, L], f32, tag="cacc",
                                            name=f"cacc{j}", bufs=2)
                            nc.vector.tensor_scalar_mul(
                                cacc[:], xcp[:, 0:L],
                                convw_sb[:, j * D_CONV:j * D_CONV + 1])
                            for i in range(1, D_CONV):
                                nc.vector.scalar_tensor_tensor(
                                    cacc[:], xcp[:, i:i + L],
                                    convw_sb[:, j * D_CONV + i:j * D_CONV + i + 1],
                                    cacc[:], Alu.mult, Alu.add)
                            nc.scalar.activation(xcs[j][:], cacc[:], Act.Silu,
                                                 bias=convb_sb[:, j:j + 1])

                    # right-side pools for wv/g0 (live P2..P4)
                    wvg_ctx = contextlib.ExitStack()
                    wv_pool = wvg_ctx.enter_context(
                        tc.tile_pool(name="wv_pool", bufs=1, side="right"))
                    wvs = [wv_pool.tile([P, L], bf16, tag=f"wv{j}", name=f"wv{j}")
                           for j in range(NJ)]
                    g0_pool = wvg_ctx.enter_context(
                        tc.tile_pool(name="g0_pool", bufs=1, side="right"))
                    g0s = [g0_pool.tile([P, L], f32, tag=f"g0{j}", name=f"g0{j}")
                           for j in range(NJ)]

                    # P2: x_proj; P3: wv/g0
                    with tc.tile_pool(name="p2t", bufs=1) as p2t:
                        for t in range(NT):
                            ps = psum.tile([P, TCH], f32, tag="ps", name=f"xproj{t}")
                            for k in range(NFB_XC):
                                lw = p2t.tile([P, P], f32r, tag="xpw",
                                              name=f"xpw{t}_{k}", bufs=2)
                                nc.sync.dma_start(lw[:], xpw_t[k])
                                nc.tensor.matmul(ps[:], lw[:],
                                                 xcs[k][:, t * TCH:(t + 1) * TCH],
                                                 start=(k == 0), stop=(k == NFB_XC - 1))
                            nc.scalar.copy(dt_sb[:, t * TCH:(t + 1) * TCH],
                                           ps[0:DT_RANK, :])
                            bcB_sb = p2t.tile([D_STATE, TCH], bf16, tag="bcB_sb",
                                              name=f"bcB_sb{t}", bufs=2)
                            nc.scalar.copy(bcB_sb[:], ps[64:80, :])
                            nc.sync.dma_start(bcB_dram[:, t * TCH:(t + 1) * TCH], bcB_sb[:])
                            bcC_sb = p2t.tile([D_STATE, TCH], f32, tag="bcC_sb",
                                              name=f"bcC_sb{t}", bufs=2)
                            nc.scalar.copy(bcC_sb[:], ps[96:112, :])
                            nc.sync.dma_start(bcC_dram[:, t * TCH:(t + 1) * TCH], bcC_sb[:])

                        for j in range(NJ):
                            dtmp = p2t.tile([P, L], f32, tag="dtmp", name=f"dtmp{j}",
                                            bufs=2)
                            dt_proj_delta(j, dtmp, p2t)
                            nc.vector.tensor_tensor(wvs[j][:], dtmp[:], xcs[j][:],
                                                    Alu.mult)
                            t1 = p2t.tile([P, L], f32, tag="g0tmp", name=f"g0tmp{j}",
                                          bufs=2)
                            nc.vector.tensor_scalar_mul(t1[:], xcs[j][:], D_sb[:, j:j + 1])
                            nc.vector.tensor_tensor(g0s[j][:], t1[:], sz[j][:], Alu.mult)

                # ---------------- P4: scan + gating ----------------
                yg_ctx = contextlib.ExitStack()
                yg_pool = yg_ctx.enter_context(tc.tile_pool(name="yg_pool", bufs=1))
                ygs = [yg_pool.tile([P, L], f32r, tag=f"yg{j}", name=f"yg{j}")
                       for j in range(NJ)]
                with tc.tile_pool(name="spool", bufs=1) as spool, \
                     tc.tile_pool(name="tpool", bufs=1) as tpool, \
                     tc.tile_pool(name="pscan", bufs=1, space="PSUM") as pscan:
                    for hb in range(4):
                        js = list(range(hb * 2, hb * 2 + 2))
                        deltas = {}
                        yps = {}
                        for j in js:
                            dj = spool.tile([P, L], f32, tag=f"delta{j % 2}",
                                            name=f"delta{j}")
                            dt_proj_delta(j, dj, spool)
                            deltas[j] = dj
                            yps[j] = pscan.tile([P, L], f32, tag=f"yps{j % 2}",
                                                name=f"yps{j}")
                        for n in range(D_STATE):
                            Bbc = tpool.tile([P, L], bf16, tag="Bbc",
                                             name=f"Bbc{hb}_{n}", bufs=2)
                            nc.sync.dma_start(
                                Bbc[:],
                                bcB_dram[n:n + 1, :].partition_broadcast(P).squeeze(1))
                            Cbc = tpool.tile([P, L], f32, tag="Cbc",
                                             name=f"Cbc{hb}_{n}", bufs=2)
                            nc.sync.dma_start(
                                Cbc[:],
                                bcC_dram[n:n + 1, :].partition_broadcast(P).squeeze(1))
                            for j in js:
                                a_t = tpool.tile([P, L], f32, tag="a_t",
                                                 name=f"a{j}_{n}", bufs=3)
                                nc.scalar.activation(
                                    a_t[:], deltas[j][:], Act.Exp,
                                    scale=A_sb[:, j * D_STATE + n:j * D_STATE + n + 1])
                                b_t = tpool.tile([P, L], bf16, tag="b_t",
                                                 name=f"b{j}_{n}", bufs=1)
                                nc.vector.tensor_tensor(b_t[:], wvs[j][:], Bbc[:],
                                                        Alu.mult)
                                h_t = tpool.tile([P, L], f32, tag="h_t",
                                                 name=f"h{j}_{n}", bufs=2)
                                nc.vector.tensor_tensor_scan(h_t[:], a_t[:], b_t[:],
                                                             0.0, Alu.mult, Alu.add)
                                prod = tpool.tile([P, L], f32r, tag="prod",
                                                  name=f"p{j}_{n}", bufs=3)
                                nc.vector.tensor_tensor(prod[:], h_t[:], Cbc[:],
                                                        Alu.mult)
                                for t in range(NT):
                                    sl = slice(t * TCH, (t + 1) * TCH)
                                    nc.tensor.matmul(yps[j][:, sl], ident_sb[:],
                                                     prod[:, sl],
                                                     start=(n == 0),
                                                     stop=(n == D_STATE - 1))
                        for j in js:
                            ygt = tpool.tile([P, L], f32, tag="ygt",
                                             name=f"ygt{j}", bufs=2)
                            nc.vector.tensor_tensor(ygt[:], yps[j][:], sz[j][:],
                                                    Alu.mult)
                            nc.vector.tensor_tensor(ygs[j][:], ygt[:], g0s[j][:],
                                                    Alu.add)
                wvg_ctx.close()  # wv + g0 (right side, LIFO: g0 then wv)

                # =========== P5: out_proj partials ===========
                with tc.tile_pool(name="p5t", bufs=1) as p5t:
                    for m in range(NJ):
                        lws = []
                        for k in range(KD):
                            lw = p5t.tile([P, P], f32r, tag=f"lw{k}",
                                          name=f"ow{m}_{k}", bufs=2)
                            nc.sync.dma_start(lw[:], outw_t[k, m])
                            lws.append(lw)
                        msb = p5t.tile([P, L], f32, tag="msb", name=f"msb{m}", bufs=2)
                        for t in range(NT):
                            ps = psum.tile([P, TCH], f32, tag="ps", name=f"op{m}_{t}")
                            mm_accum(ps, lws,
                                     lambda k: ygs[k][:, t * TCH:(t + 1) * TCH], KD)
                            nc.scalar.copy(msb[:, t * TCH:(t + 1) * TCH], ps[:])
                        nc.sync.dma_start(ag_in[m * P:(m + 1) * P, :], msb[:])
                yg_ctx.close()

            nc.gpsimd.collective_compute("AllGather", Alu.bypass,
                                         replica_groups=_GROUPS,
                                         ins=[ag_in[:]], outs=[ag_out[:]])

            # =========== P6: mo + LN ===========
            with contextlib.ExitStack() as stk2:
                mo_pool = stk2.enter_context(tc.tile_pool(name="mo_pool", bufs=1))
                mos = [mo_pool.tile([P, L], f32r, tag=f"mo{j}", name=f"mo{j}")
                       for j in range(KD)]
                xn_pool = stk2.enter_context(tc.tile_pool(name="xn_pool", bufs=1))
                xns = [xn_pool.tile([P, L], f32r, tag=f"xn{j}", name=f"xn{j}")
                       for j in range(KD)]

                with tc.tile_pool(name="p6t", bufs=1) as p6t, \
                     tc.tile_pool(name="pln", bufs=1, space="PSUM") as pln:
                    mu_ps = pln.tile([1, L], f32, tag="mu_ps", name="mu_ps", bufs=1)
                    e2_ps = pln.tile([1, L], f32, tag="e2_ps", name="e2_ps", bufs=1)
                    for j in range(KD):
                        parts = []
                        for q in range(4):
                            pt = p6t.tile([P, L], f32, tag="agp", name=f"agp{j}_{q}",
                                          bufs=4)
                            nc.sync.dma_start(
                                pt[:],
                                ag_out[q * D_MODEL + j * P:q * D_MODEL + (j + 1) * P, :])
                            parts.append(pt)
                        a01 = p6t.tile([P, L], f32, tag="a01", name=f"a01_{j}", bufs=2)
                        nc.vector.tensor_tensor(a01[:], parts[0][:], parts[1][:],
                                                Alu.add)
                        a23 = p6t.tile([P, L], f32, tag="a23", name=f"a23_{j}", bufs=2)
                        nc.vector.tensor_tensor(a23[:], parts[2][:], parts[3][:],
                                                Alu.add)
                        nc.vector.tensor_tensor(mos[j][:], a01[:], a23[:, ::-1],
                                                Alu.add)
                        sq = p6t.tile([P, L], f32r, tag="sq", name=f"sq{j}", bufs=2)
                        nc.scalar.activation(sq[:], mos[j][:], Act.Square)
                        for t in range(NT):
                            sl = slice(t * TCH, (t + 1) * TCH)
                            nc.tensor.matmul(mu_ps[:, sl], ones_sb[:],
                                             mos[j][:, sl],
                                             start=(j == 0), stop=(j == KD - 1))
                            nc.tensor.matmul(e2_ps[:, sl], ones_sb[:],
                                             sq[:, sl],
                                             start=(j == 0), stop=(j == KD - 1))

                    mean_sb = p6t.tile([1, L], f32r, tag="mean_sb", name="mean_sb",
                                       bufs=1)
                    nc.scalar.copy(mean_sb[:], mu_ps[:])
                    m2 = p6t.tile([1, L], f32, tag="m2", name="m2", bufs=1)
                    nc.vector.tensor_tensor(m2[:], mean_sb[:], mean_sb[:], Alu.mult)
                    var_t = p6t.tile([1, L], f32, tag="var_t", name="var_t", bufs=1)
                    nc.vector.tensor_tensor(var_t[:], e2_ps[:], m2[:], Alu.subtract)
                    eps_sb = p6t.tile([1, 1], f32, tag="eps_sb", name="eps_sb", bufs=1)
                    nc.vector.memset(eps_sb[:], 1e-5)
                    std_t = p6t.tile([1, L], f32, tag="std_t", name="std_t", bufs=1)
                    nc.scalar.activation(std_t[:], var_t[:], Act.Sqrt, bias=eps_sb[:])
                    rstd_sb = p6t.tile([1, L], f32r, tag="rstd_sb", name="rstd_sb",
                                       bufs=1)
                    with nc.allow_low_precision(reason="f32r view of fp32 rstd"):
                        nc.vector.reciprocal(rstd_sb[:], std_t[:])
                    onesrow = p6t.tile([1, P], f32r, tag="onesrow", name="onesrow",
                                       bufs=1)
                    nc.scalar.activation(onesrow[:], ident_sb[0:1, :], Act.Copy,
                                         bias=1.0, scale=0.0)
                    mean_bc = pln.tile([P, L], f32, tag="mu_ps", name="mean_bc",
                                       bufs=1)
                    rstd_bc = pln.tile([P, L], f32, tag="e2_ps", name="rstd_bc",
                                       bufs=1)
                    for t in range(NT):
                        sl = slice(t * TCH, (t + 1) * TCH)
                        nc.tensor.matmul(mean_bc[:, sl], onesrow[:], mean_sb[:, sl],
                                         start=True, stop=True)
                        nc.tensor.matmul(rstd_bc[:, sl], onesrow[:], rstd_sb[:, sl],
                                         start=True, stop=True)

                    for j in range(KD):
                        t1 = p6t.tile([P, L], f32, tag="lnt", name=f"lnt{j}", bufs=2)
                        nc.vector.tensor_tensor(t1[:], mos[j][:], mean_bc[:],
                                                Alu.subtract)
                        nc.vector.tensor_tensor(t1[:], t1[:], rstd_bc[:], Alu.mult)
                        nc.vector.tensor_scalar(xns[j][:], t1[:], lng_sb[:, j:j + 1],
                                                lnb_sb[:, j:j + 1], Alu.mult, Alu.add)

                # =========== P7: FFN ===========
                with tc.tile_pool(name="ffh_pool", bufs=1) as ffh_pool, \
                     tc.tile_pool(name="p7t", bufs=1) as p7t:
                    ffhs = [ffh_pool.tile([P, L], f32r, tag=f"ffh{m}", name=f"ffh{m}")
                            for m in range(NJ)]
                    for m in range(NJ):
                        lws = []
                        for k in range(KD):
                            lw = p7t.tile([P, P], f32r, tag=f"lw{k}", name=f"w1_{m}_{k}",
                                          bufs=2)
                            nc.sync.dma_start(lw[:], w1_t[k, m])
                            lws.append(lw)
                        for t in range(NT):
                            ps = psum.tile([P, TCH], f32, tag="ps", name=f"f1{m}_{t}")
                            mm_accum(ps, lws,
                                     lambda k: xns[k][:, t * TCH:(t + 1) * TCH], KD)
                            nc.scalar.activation(ffhs[m][:, t * TCH:(t + 1) * TCH],
                                                 ps[:], Act.Gelu,
                                                 bias=b1_sb[:, m:m + 1])

                    for m in range(KD):
                        lws = []
                        for k in range(NJ):
                            lw = p7t.tile([P, P], f32r, tag=f"lw{k}", name=f"w2_{m}_{k}",
                                          bufs=2)
                            nc.sync.dma_start(lw[:], w2_t[k, m])
                            lws.append(lw)
                        msb = p7t.tile([P, L], f32, tag="msb", name=f"f2sb{m}", bufs=2)
                        for t in range(NT):
                            ps = psum.tile([P, TCH], f32, tag="ps", name=f"f2{m}_{t}")
                            mm_accum(ps, lws,
                                     lambda k: ffhs[k][:, t * TCH:(t + 1) * TCH], NJ)
                            nc.scalar.copy(msb[:, t * TCH:(t + 1) * TCH], ps[:])
                        nc.sync.dma_start(ar_in[m * P:(m + 1) * P, :], msb[:])

            nc.gpsimd.collective_compute("AllReduce", Alu.add, replica_groups=_GROUPS,
                                         ins=[ar_in[:]], outs=[ar_out[:]])

            with tc.tile_pool(name="p8t", bufs=1) as p8t:
                for j in range(KD):
                    fin = p8t.tile([P, L], f32, tag="fin", name=f"fin{j}", bufs=2)
                    nc.sync.dma_start(fin[:], ar_out[j * P:(j + 1) * P, :])
                    fob = p8t.tile([P, L], f32, tag="fob", name=f"fob{j}", bufs=2)
                    nc.vector.tensor_scalar_add(fob[:], fin[:], b2_sb[:, j:j + 1])
                    nc.sync.dma_start(out_m[j * P:(j + 1) * P, :], fob[:])

    nc.compile()
    return nc


def _prep_inputs(inputs):
    """Per-core input dicts. Core c: sequence s=c//2 (s>=2 => time-flipped x),
    d_inner half = c%2. The own half of d_inner is permuted FIRST in every
    d_inner-ordered tensor, so the device kernel is identical on all cores."""
    x = np.asarray(inputs["x"], dtype=np.float32)
    in_proj_w = np.asarray(inputs["in_proj_w"], dtype=np.float32)
    conv_w = np.asarray(inputs["conv_w"], dtype=np.float32)
    conv_b = np.asarray(inputs["conv_b"], dtype=np.float32)
    x_proj_w = np.asarray(inputs["x_proj_w"], dtype=np.float32)
    dt_proj_w = np.asarray(inputs["dt_proj_w"], dtype=np.float32)
    dt_proj_b = np.asarray(inputs["dt_proj_b"], dtype=np.float32)
    A = -np.exp(np.asarray(inputs["A_log"], dtype=np.float32))
    Dp = np.asarray(inputs["D"], dtype=np.float32)
    out_proj_w = np.asarray(inputs["out_proj_w"], dtype=np.float32)
    ln_g = np.asarray(inputs["ln_g"], dtype=np.float32)
    ln_b = np.asarray(inputs["ln_b"], dtype=np.float32)
    ff_w1 = np.asarray(inputs["ff_w1"], dtype=np.float32)
    ff_b1 = np.asarray(inputs["ff_b1"], dtype=np.float32)
    ff_w2 = np.asarray(inputs["ff_w2"], dtype=np.float32)
    ff_b2 = np.asarray(inputs["ff_b2"], dtype=np.float32)

    def cols(v):  # (N,) -> (P, N//P) per-partition column layout
        return np.ascontiguousarray(v.reshape(-1, P).T)

    def tile_w(w, KP, MP):  # (K, M) -> (K//KP, M//MP, KP, MP)
        K, M = w.shape
        return np.ascontiguousarray(
            w.reshape(K // KP, KP, M // MP, MP).transpose(0, 2, 1, 3))

    in_maps = []
    for c in range(8):
        s, half = c // 2, c % 2
        xb = x[s] if s < 2 else x[s - 2][::-1]
        perm = np.arange(D_INNER).reshape(2, HALF)
        perm = np.concatenate([perm[half], perm[1 - half]])
        own = perm[:HALF]

        wz = in_proj_w[:, D_INNER + own]                      # (1024, 1024)
        wxc = in_proj_w[:, perm]                              # (1024, 2048)
        w_in = np.concatenate([wz, wxc], axis=1)              # (1024, 3072)
        w_in_t = np.ascontiguousarray(tile_w(w_in, P, P).transpose(1, 0, 2, 3))

        cw = conv_w[perm]  # (2048, 4) -> (P, 16*4): col j*4+i = w[jP+p, i]
        convw_cols = np.ascontiguousarray(
            cw.reshape(NFB_XC, P, D_CONV).transpose(1, 0, 2).reshape(P, NFB_XC * D_CONV))

        g = (c & 1) + 2 * (c >> 2)
        hsl = slice(g * FF_SLICE, (g + 1) * FF_SLICE)

        in_maps.append({
            "xT": np.ascontiguousarray(xb.T),
            "w_in_t": w_in_t,
            "convw_cols": convw_cols,
            "convb_cols": cols(conv_b[perm]),
            "xpw_t": np.ascontiguousarray(
                np.concatenate([
                    x_proj_w[perm][:, :DT_RANK + D_STATE],
                    np.zeros((D_INNER, D_STATE), np.float32),
                    x_proj_w[perm][:, DT_RANK + D_STATE:],
                    np.zeros((D_INNER, D_STATE), np.float32),
                ], axis=1).reshape(NFB_XC, P, P)),
            "dtw_t": np.ascontiguousarray(
                dt_proj_w[:, own].reshape(DT_RANK, NJ, P).transpose(1, 0, 2)),
            "dtb_cols": cols(dt_proj_b[own]),
            "A_cols": np.ascontiguousarray(
                A[own].reshape(NJ, P, D_STATE).transpose(1, 0, 2).reshape(P, NJ * D_STATE)),
            "D_colsT": cols(Dp[own]),
            "outw_t": tile_w(out_proj_w[own], P, P),
            "lng_cols": cols(ln_g),
            "lnb_cols": cols(ln_b),
            "w1_t": tile_w(ff_w1[:, hsl], P, P),
            "b1_cols": cols(ff_b1[hsl]),
            "w2_t": tile_w(ff_w2[hsl], P, P),
            "b2_cols": cols(ff_b2),
            "ident_r": np.eye(P, dtype=np.float32),
            "consts_r": np.concatenate(
                [np.full((P, 1), 1.0 / D_MODEL, np.float32),
                 np.zeros((P, 3), np.float32)], axis=1),
        })
    return in_maps


_NC_CACHE = {}


def _get_nc():
    if "nc" not in _NC_CACHE:
        _NC_CACHE["nc"] = _build_nc()
    return _NC_CACHE["nc"]


def run(inputs, trace=False):
    _install_ntff_hook_shim()
    from concourse import bass_utils
    nc = _get_nc()
    in_maps = _prep_inputs(inputs)
    res = bass_utils.run_bass_kernel_spmd(nc, in_maps, core_ids=list(range(8)),
                                          trace=trace)
    out = np.stack([
        np.ascontiguousarray(res.results[0]["out_m"].T),
        np.ascontiguousarray(res.results[2]["out_m"].T),
    ]).astype(np.float32)
    return out, res


def kernel(**inputs):
    out, _ = run(inputs, trace=False)
    return out
